# revision 53
# baseline (speedup 1.0000x reference)
"""Trainium2 Bass kernel for nn_AFM (attentional factorization machine).

Mathematical reduction (validated against the reference):
  - softmax over a size-1 axis == 1, so the attention MLP is dead code and
    fAtt = mean(fPI, axis=1).
  - FM identity per (b, m): sum_{i<j} x_i x_j = ((sum_i x_i)^2 - sum_i x_i^2)/2
    with x_i = dense[b,i,m] * v[i,m].
  With S1[b,m] = sum_n dense[b,n,m] v[n,m], S2[b,m] = sum_n (dense[b,n,m] v[n,m])^2,
  c[m] = Wp[m] / (2 * P):
    out[b] = sum_n dense[b,n,0] Wl[n] + bl + bp + sum_m c[m] (S1[b,m]^2 - S2[b,m])

Sharding: pure data parallel, batch 4096 -> 512 rows on each of 8 cores.

Raw-bass SPMD program (no Tile framework; manual semaphores) per core,
software-pipelined over four 128-row tiles, each loaded in two half-tiles:
  SYNC: HWDGE f32 half-tile loads + small param loads; one batched [128,4]
        output store at the end (host transposes to batch order).
  DVE:  per tile: dva/dvb = d*v on each half (f32 in, bf16 out), S1 log-tree
        (level 1 sums the halves), then - pipelined one tile behind - the S2
        log-tree over the squared halves and the fused combine chain
        (custom-DVE tensor-tensor-reduce). GpSimd is deliberately unused:
        concurrent GpSimd traffic knocks DVE tensor ops out of their 2x mode.
  ACT:  per tile: square(dva), square(dvb) -> bf16 halves for the S2 tree.
Cross-engine ordering uses per-engine chain semaphores; every compute
instruction waits on its chain and increments it. Cross-engine wait
thresholds are chosen so each semaphore value has a unique producer
(required by the race checker); WAIT_OVERRIDES carries sim-calibrated
adjustments.
"""

import numpy as np

B, N, M = 4096, 32, 64
NM = N * M                  # 2048
HALF = NM // 2              # 1024 (= n in [0,16) block)
NCORES = 8
BS = B // NCORES            # 512 rows per core
TILES = BS // 128           # 4 tiles of 128 batch rows per core
P_PAIRS = N * (N - 1) // 2  # 496

# tree level output widths: 1024 (sum of halves), then halving to 64
LVLS = [1024, 512, 256, 128, 64]

_CACHE = {}

WAIT_OVERRIDES = {('vch', 5): 6}  # sim-calibrated


def _build_program():
    from concourse import bacc, mybir
    from concourse.dve_ops import TENSOR_TENSOR_REDUCE as CTTR

    f32 = mybir.dt.float32
    bf16 = mybir.dt.bfloat16

    nc = bacc.Bacc("TRN2", target_bir_lowering=False, debug=False)
    dense = nc.declare_dram_parameter("dense", [BS, NM], f32, isOutput=False)
    vrep = nc.declare_dram_parameter("vrep", [128, NM], bf16, isOutput=False)
    crep = nc.declare_dram_parameter("crep", [128, M], f32, isOutput=False)
    wlrep = nc.declare_dram_parameter("wlrep", [128, N], f32, isOutput=False)
    cst = nc.declare_dram_parameter("cst", [128, 1], f32, isOutput=False)
    out = nc.declare_dram_parameter("out", [128, TILES], f32, isOutput=True)

    sb = lambda name, shape, dt: nc.alloc_sbuf_tensor(name, list(shape), dt)

    vrep_t = sb("vrep_t", [128, NM], bf16)
    crep_t = sb("crep_t", [128, M], f32)
    wlrep_t = sb("wlrep_t", [128, N], f32)
    cst_t = sb("cst_t", [128, 1], f32)
    o2all = sb("o2all", [128, TILES], f32)

    # ds*_t hold [dv | sq] side by side: DVE writes cols [0:HALF] (d*v),
    # ACT writes cols [HALF:2*HALF] (square of the dv half). The two
    # reduction trees then run as combined ops over both halves.
    df_t, dsa_t, dsb_t = [], [], []
    s12lv = []
    cs1_t, junkM, junkN, pc1_t, pc2_t = [], [], [], [], []
    for t in range(TILES):
        df_t.append(sb(f"df{t}", [128, NM], f32))
        dsa_t.append(sb(f"dsa{t}", [128, NM], bf16))
        dsb_t.append(sb(f"dsb{t}", [128, NM], bf16))
        s12lv.append(
            [sb(f"s12_{t}_{w}", [128, 2 * w], f32 if w == M else bf16) for w in LVLS]
        )
        cs1_t.append(sb(f"cs1_{t}", [128, M], f32))
        junkM.append(sb(f"junkM_{t}", [128, M], f32))
        junkN.append(sb(f"junkN_{t}", [128, N], f32))
        pc1_t.append(sb(f"pc1_{t}", [128, 1], f32))
        pc2_t.append(sb(f"pc2_{t}", [128, 1], f32))

    def tree_step(eng, t, lvl):
        """One combined halving add over both trees ([s1-block | s2-block])."""
        levels = s12lv[t]
        w = LVLS[lvl]
        if lvl == 0:
            return eng.tensor_add(levels[0].ap(), dsa_t[t].ap(), dsb_t[t].ap())
        src = levels[lvl - 1].ap().rearrange("p (s w) -> p s w", s=2)
        return eng.tensor_add(
            levels[lvl].ap().rearrange("p (s w) -> p s w", s=2),
            src[:, :, 0:w],
            src[:, :, w : 2 * w],
        )

    cnt = {"v": 0, "a": 0, "s": 0}
    chains = {}

    def emit(e, ins):
        ins._wait_ge(chains[e], cnt[e]).then_inc(chains[e], 1)
        cnt[e] += 1
        return cnt[e]

    def emit_dma(e, ins, sem, inc, wait=None):
        if wait is not None:
            wsem, wval = wait
            wval = WAIT_OVERRIDES.get((wsem.name, wval), wval)
            ins._wait_ge(wsem, wval)
        else:
            ins._wait_ge(chains[e], cnt[e])
        ins.then_inc(sem, inc)

    def emit_wait(e, eng, sem, val):
        val = WAIT_OVERRIDES.get((sem.name, val), val)
        eng.wait_ge(sem, val).then_inc(chains[e], 1)
        cnt[e] += 1

    dv_done = [0] * TILES      # vchain value after dvb of tile t
    sq_done = [0] * TILES      # achain value after sqb of tile t
    s2first_done = [0] * TILES # vchain value after first s2 tree op of tile t
    o2_done = [0] * TILES      # vchain value after final combine of tile t

    with (
        nc.Block() as block,
        nc.semaphore("vch") as vch,
        nc.semaphore("ach") as ach,
        nc.semaphore("sch") as sch,
        nc.semaphore("ld0a") as ld0a,
        nc.semaphore("ld0b") as ld0b,
        nc.semaphore("ld1a") as ld1a,
        nc.semaphore("ld1b") as ld1b,
        nc.semaphore("ld2a") as ld2a,
        nc.semaphore("ld2b") as ld2b,
        nc.semaphore("ld3a") as ld3a,
        nc.semaphore("ld3b") as ld3b,
        nc.semaphore("vr") as vr,
        nc.semaphore("prm") as prm,
        nc.semaphore("sts") as sts,
    ):
        chains.update(v=vch, a=ach, s=sch)
        lda = [ld0a, ld1a, ld2a, ld3a]
        ldb = [ld0b, ld1b, ld2b, ld3b]

        @block.vector
        def _(dve):
            def head(t):
                emit_wait("v", dve, lda[t], 16)
                emit("v", dve.tensor_mul(
                    dsa_t[t].ap()[:, 0:HALF], df_t[t].ap()[:, 0:HALF],
                    vrep_t.ap()[:, 0:HALF],
                ))
                emit_wait("v", dve, ldb[t], 16)
                dv_done[t] = emit("v", dve.tensor_mul(
                    dsb_t[t].ap()[:, 0:HALF], df_t[t].ap()[:, HALF:NM],
                    vrep_t.ap()[:, HALF:NM],
                ))

            def tail(t):
                # ach counts 3 per tile (wait, sqa, sqb); scalar block is
                # built after this one so sq_done[t] isn't available yet
                emit_wait("v", dve, ach, 3 * (t + 1))
                s2first_done[t] = cnt["v"] + 1  # vch value of combined L1
                for lvl in range(len(LVLS)):
                    emit("v", tree_step(dve, t, lvl))
                s1f = s12lv[t][-1].ap()[:, 0:M]
                s2f = s12lv[t][-1].ap()[:, M : 2 * M]
                emit("v", dve.tensor_mul(cs1_t[t].ap(), s1f, crep_t.ap()))
                emit("v", dve._custom_dve(
                    CTTR, out=junkM[t].ap(), in0=cs1_t[t].ap(),
                    in1=s1f, s0=cst_t.ap(), s1=1.0,
                    accum_out=pc1_t[t].ap(),
                ))
                emit("v", dve._custom_dve(
                    CTTR, out=junkM[t].ap(), in0=s2f,
                    in1=crep_t.ap(), s0=pc1_t[t].ap(), s1=-1.0,
                    accum_out=pc2_t[t].ap(),
                ))
                d_col0 = (
                    df_t[t]
                    .ap()
                    .rearrange("p (n m) -> p n m", n=N)[:, :, 0:1]
                    .rearrange("p n one -> p (n one)")
                )
                o2_done[t] = emit("v", dve._custom_dve(
                    CTTR, out=junkN[t].ap(), in0=d_col0, in1=wlrep_t.ap(),
                    s0=pc2_t[t].ap(), s1=1.0,
                    accum_out=o2all.ap()[:, t : t + 1],
                ))

            # heads (multiplies) interleave with tails (trees+combine),
            # hiding the square latency behind the next tile's multiplies
            emit_wait("v", dve, vr, 16)
            head(0)
            head(1)
            emit_wait("v", dve, prm, 48)
            tail(0)
            head(2)
            tail(1)
            head(3)
            tail(2)
            tail(3)

        @block.scalar
        def _(act):
            # param loads ride the Activation HWDGE ring so they don't
            # compete with the dense loads on the SP ring
            emit_dma("a", act.dma_start(out=vrep_t.ap(), in_=vrep.ap()), vr, 16)
            emit_dma("a", act.dma_start(out=crep_t.ap(), in_=crep.ap()), prm, 16)
            emit_dma("a", act.dma_start(out=wlrep_t.ap(), in_=wlrep.ap()), prm, 16)
            emit_dma("a", act.dma_start(out=cst_t.ap(), in_=cst.ap()), prm, 16)
            for t in range(TILES):
                # Pin the ach increment order: gate on tail(t-1)'s combined
                # L1 (which requires sq(t-1)); it also covers dv_t since
                # tail(t-1) follows head(t) in the DVE stream. t=0 gates on
                # its own dvb.
                thr = dv_done[t] if t == 0 else s2first_done[t - 1]
                emit_wait("a", act, vch, thr)
                emit("a", act.square(
                    dsa_t[t].ap()[:, HALF:NM], dsa_t[t].ap()[:, 0:HALF]
                ))
                sq_done[t] = emit("a", act.square(
                    dsb_t[t].ap()[:, HALF:NM], dsb_t[t].ap()[:, 0:HALF]
                ))

        @block.sync
        def _(sync):
            def ld(t, h):
                lo, hi = (0, HALF) if h == 0 else (HALF, NM)
                emit_dma(
                    "s",
                    sync.dma_start(
                        out=df_t[t].ap()[:, lo:hi],
                        in_=dense.ap()[128 * t : 128 * (t + 1), lo:hi],
                    ),
                    (lda if h == 0 else ldb)[t], 16,
                )

            # throttle: only one tile's loads queued at a time, so each
            # load's completion semaphore fires as soon as its data lands
            # (a deep queue round-robins packets and delays the first
            # completion to nearly the last)
            ld(0, 0)
            ld(0, 1)
            for t in range(1, TILES):
                emit_wait("s", sync, lda[t - 1], 16)
                ld(t, 0)
                ld(t, 1)
            emit_dma(
                "s",
                sync.dma_start(out=out.ap(), in_=o2all.ap()),
                sts, 16,
                wait=(vch, o2_done[3]),
            )
            sync.wait_ge(sts, 16)

    nc.compile()
    return nc


def _get_program():
    if "nc" not in _CACHE:
        _CACHE["nc"] = _build_program()
    return _CACHE["nc"]


def _host_prep(inputs):
    dense = np.ascontiguousarray(
        np.asarray(inputs["dense"], dtype=np.float32).reshape(B, NM)
    )
    v = np.asarray(inputs["v"], dtype=np.float32).reshape(1, NM)
    Wl = np.asarray(inputs["Wl"], dtype=np.float32).reshape(N)
    Wp = np.asarray(inputs["Wp"], dtype=np.float32).reshape(M)
    bl = float(np.asarray(inputs["bl"], dtype=np.float32).reshape(-1)[0])
    bp = float(np.asarray(inputs["bp"], dtype=np.float32).reshape(-1)[0])

    import ml_dtypes

    c = (Wp / (2.0 * P_PAIRS)).astype(np.float32)
    vrep = np.ascontiguousarray(
        np.broadcast_to(v.astype(ml_dtypes.bfloat16), (128, NM))
    )
    crep = np.ascontiguousarray(np.broadcast_to(c[None, :], (128, M)))
    wlrep = np.ascontiguousarray(np.broadcast_to(Wl[None, :], (128, N)))
    cst = np.full((128, 1), bl + bp, dtype=np.float32)

    in_maps = []
    for i in range(NCORES):
        in_maps.append(
            {
                "dense": dense[BS * i : BS * (i + 1)],
                "vrep": vrep,
                "crep": crep,
                "wlrep": wlrep,
                "cst": cst,
            }
        )
    return in_maps


def _gather(res):
    # out[p, t] holds batch row 128*t + p of the core's shard
    outs = []
    for i in range(NCORES):
        arr = np.asarray(res.results[i]["out"], np.float32)  # [128, TILES]
        outs.append(arr.T.reshape(BS))
    return np.concatenate(outs).reshape(B, 1)


def kernel(**inputs) -> np.ndarray:
    from concourse.bass_utils import run_bass_kernel_spmd

    nc = _get_program()
    in_maps = _host_prep(inputs)
    res = run_bass_kernel_spmd(nc, in_maps, core_ids=list(range(NCORES)))
    return _gather(res)


# revision 54
# speedup vs baseline: 1.0257x; 1.0257x over previous
"""Trainium2 Bass kernel for nn_AFM (attentional factorization machine).

Mathematical reduction (validated against the reference):
  - softmax over a size-1 axis == 1, so the attention MLP is dead code and
    fAtt = mean(fPI, axis=1).
  - FM identity per (b, m): sum_{i<j} x_i x_j = ((sum_i x_i)^2 - sum_i x_i^2)/2
    with x_i = dense[b,i,m] * v[i,m].
  With S1[b,m] = sum_n dense[b,n,m] v[n,m], S2[b,m] = sum_n (dense[b,n,m] v[n,m])^2,
  c[m] = Wp[m] / (2 * P):
    out[b] = sum_n dense[b,n,0] Wl[n] + bl + bp + sum_m c[m] (S1[b,m]^2 - S2[b,m])

Sharding: pure data parallel, batch 4096 -> 512 rows on each of 8 cores.

Raw-bass SPMD program (no Tile framework; manual semaphores) per core,
software-pipelined over four 128-row tiles, each loaded in two half-tiles:
  SYNC: HWDGE f32 half-tile loads + small param loads; one batched [128,4]
        output store at the end (host transposes to batch order).
  DVE:  per tile: dva/dvb = d*v on each half (f32 in, bf16 out), S1 log-tree
        (level 1 sums the halves), then - pipelined one tile behind - the S2
        log-tree over the squared halves and the fused combine chain
        (custom-DVE tensor-tensor-reduce). GpSimd is deliberately unused:
        concurrent GpSimd traffic knocks DVE tensor ops out of their 2x mode.
  ACT:  per tile: square(dva), square(dvb) -> bf16 halves for the S2 tree.
Cross-engine ordering uses per-engine chain semaphores; every compute
instruction waits on its chain and increments it. Cross-engine wait
thresholds are chosen so each semaphore value has a unique producer
(required by the race checker); WAIT_OVERRIDES carries sim-calibrated
adjustments.
"""

import numpy as np

B, N, M = 4096, 32, 64
NM = N * M                  # 2048
HALF = NM // 2              # 1024 (= n in [0,16) block)
NCORES = 8
BS = B // NCORES            # 512 rows per core
TILES = BS // 128           # 4 tiles of 128 batch rows per core
P_PAIRS = N * (N - 1) // 2  # 496

# tree level output widths: 1024 (sum of halves), then halving to 64
LVLS = [1024, 512, 256, 128, 64]

_CACHE = {}

WAIT_OVERRIDES = {('vch', 5): 6}  # sim-calibrated


def _build_program():
    from concourse import bacc, mybir
    from concourse.dve_ops import TENSOR_TENSOR_REDUCE as CTTR

    f32 = mybir.dt.float32
    bf16 = mybir.dt.bfloat16

    nc = bacc.Bacc("TRN2", target_bir_lowering=False, debug=False)
    dense = nc.declare_dram_parameter("dense", [BS, NM], f32, isOutput=False)
    vrep = nc.declare_dram_parameter("vrep", [128, NM], bf16, isOutput=False)
    crep = nc.declare_dram_parameter("crep", [128, M], f32, isOutput=False)
    wlrep = nc.declare_dram_parameter("wlrep", [128, N], f32, isOutput=False)
    cst = nc.declare_dram_parameter("cst", [128, 1], f32, isOutput=False)
    out = nc.declare_dram_parameter("out", [128, TILES], f32, isOutput=True)

    sb = lambda name, shape, dt: nc.alloc_sbuf_tensor(name, list(shape), dt)

    vrep_t = sb("vrep_t", [128, NM], bf16)
    crep_t = sb("crep_t", [128, M], f32)
    wlrep_t = sb("wlrep_t", [128, N], f32)
    cst_t = sb("cst_t", [128, 1], f32)
    o2all = sb("o2all", [128, TILES], f32)

    # ds*_t hold [dv | sq] side by side: DVE writes cols [0:HALF] (d*v),
    # ACT writes cols [HALF:2*HALF] (square of the dv half). The two
    # reduction trees then run as combined ops over both halves.
    df_t, dsa_t, dsb_t = [], [], []
    s12lv = []
    cs1_t, junkM, junkN, pc1_t, pc2_t = [], [], [], [], []
    for t in range(TILES):
        df_t.append(sb(f"df{t}", [128, NM], f32))
        dsa_t.append(sb(f"dsa{t}", [128, NM], bf16))
        dsb_t.append(sb(f"dsb{t}", [128, NM], bf16))
        s12lv.append(
            [sb(f"s12_{t}_{w}", [128, 2 * w], f32 if w == M else bf16) for w in LVLS]
        )
        cs1_t.append(sb(f"cs1_{t}", [128, M], f32))
        junkM.append(sb(f"junkM_{t}", [128, M], f32))
        junkN.append(sb(f"junkN_{t}", [128, N], f32))
        pc1_t.append(sb(f"pc1_{t}", [128, 1], f32))
        pc2_t.append(sb(f"pc2_{t}", [128, 1], f32))

    def tree_step(eng, t, lvl):
        """One combined halving add over both trees ([s1-block | s2-block])."""
        levels = s12lv[t]
        w = LVLS[lvl]
        if lvl == 0:
            return eng.tensor_add(levels[0].ap(), dsa_t[t].ap(), dsb_t[t].ap())
        src = levels[lvl - 1].ap().rearrange("p (s w) -> p s w", s=2)
        return eng.tensor_add(
            levels[lvl].ap().rearrange("p (s w) -> p s w", s=2),
            src[:, :, 0:w],
            src[:, :, w : 2 * w],
        )

    cnt = {"v": 0, "a": 0, "s": 0}
    chains = {}

    def emit(e, ins):
        ins._wait_ge(chains[e], cnt[e]).then_inc(chains[e], 1)
        cnt[e] += 1
        return cnt[e]

    def emit_dma(e, ins, sem, inc, wait=None):
        if wait is not None:
            wsem, wval = wait
            wval = WAIT_OVERRIDES.get((wsem.name, wval), wval)
            ins._wait_ge(wsem, wval)
        else:
            ins._wait_ge(chains[e], cnt[e])
        ins.then_inc(sem, inc)

    def emit_wait(e, eng, sem, val):
        val = WAIT_OVERRIDES.get((sem.name, val), val)
        eng.wait_ge(sem, val).then_inc(chains[e], 1)
        cnt[e] += 1

    dv_done = [0] * TILES      # vchain value after dvb of tile t
    sq_done = [0] * TILES      # achain value after sqb of tile t
    s2first_done = [0] * TILES # vchain value after first s2 tree op of tile t
    o2_done = [0] * TILES      # vchain value after final combine of tile t

    with (
        nc.Block() as block,
        nc.semaphore("vch") as vch,
        nc.semaphore("ach") as ach,
        nc.semaphore("sch") as sch,
        nc.semaphore("ld0a") as ld0a,
        nc.semaphore("ld0b") as ld0b,
        nc.semaphore("ld1a") as ld1a,
        nc.semaphore("ld1b") as ld1b,
        nc.semaphore("ld2a") as ld2a,
        nc.semaphore("ld2b") as ld2b,
        nc.semaphore("ld3a") as ld3a,
        nc.semaphore("ld3b") as ld3b,
        nc.semaphore("vr") as vr,
        nc.semaphore("prm") as prm,
        nc.semaphore("sts") as sts,
    ):
        chains.update(v=vch, a=ach, s=sch)
        lda = [ld0a, ld1a, ld2a, ld3a]
        ldb = [ld0b, ld1b, ld2b, ld3b]

        @block.vector
        def _(dve):
            def head(t):
                emit_wait("v", dve, lda[t], 16)
                emit("v", dve.tensor_mul(
                    dsa_t[t].ap()[:, 0:HALF], df_t[t].ap()[:, 0:HALF],
                    vrep_t.ap()[:, 0:HALF],
                ))
                emit_wait("v", dve, ldb[t], 16)
                dv_done[t] = emit("v", dve.tensor_mul(
                    dsb_t[t].ap()[:, 0:HALF], df_t[t].ap()[:, HALF:NM],
                    vrep_t.ap()[:, HALF:NM],
                ))

            def tail(t):
                # ach counts 3 per tile (wait, sqa, sqb); scalar block is
                # built after this one so sq_done[t] isn't available yet
                emit_wait("v", dve, ach, 3 * (t + 1))
                s2first_done[t] = cnt["v"] + 1  # vch value of combined L1
                for lvl in range(len(LVLS)):
                    emit("v", tree_step(dve, t, lvl))
                s1f = s12lv[t][-1].ap()[:, 0:M]
                s2f = s12lv[t][-1].ap()[:, M : 2 * M]
                emit("v", dve.tensor_mul(cs1_t[t].ap(), s1f, crep_t.ap()))
                emit("v", dve._custom_dve(
                    CTTR, out=junkM[t].ap(), in0=cs1_t[t].ap(),
                    in1=s1f, s0=cst_t.ap(), s1=1.0,
                    accum_out=pc1_t[t].ap(),
                ))
                emit("v", dve._custom_dve(
                    CTTR, out=junkM[t].ap(), in0=s2f,
                    in1=crep_t.ap(), s0=pc1_t[t].ap(), s1=-1.0,
                    accum_out=pc2_t[t].ap(),
                ))
                d_col0 = (
                    df_t[t]
                    .ap()
                    .rearrange("p (n m) -> p n m", n=N)[:, :, 0:1]
                    .rearrange("p n one -> p (n one)")
                )
                o2_done[t] = emit("v", dve._custom_dve(
                    CTTR, out=junkN[t].ap(), in0=d_col0, in1=wlrep_t.ap(),
                    s0=pc2_t[t].ap(), s1=1.0,
                    accum_out=o2all.ap()[:, t : t + 1],
                ))

            # heads (multiplies) interleave with tails (trees+combine),
            # hiding the square latency behind the next tile's multiplies
            emit_wait("v", dve, vr, 16)
            head(0)
            head(1)
            emit_wait("v", dve, prm, 48)
            tail(0)
            head(2)
            tail(1)
            head(3)
            tail(2)
            tail(3)

        @block.scalar
        def _(act):
            # param loads ride the Activation HWDGE ring so they don't
            # compete with the dense loads on the SP ring
            emit_dma("a", act.dma_start(out=vrep_t.ap(), in_=vrep.ap()), vr, 16)
            emit_dma(
                "a",
                act.dma_start(
                    out=df_t[0].ap()[:, HALF:NM], in_=dense.ap()[0:128, HALF:NM]
                ),
                ldb[0], 16,
            )
            emit_dma("a", act.dma_start(out=crep_t.ap(), in_=crep.ap()), prm, 16)
            emit_dma("a", act.dma_start(out=wlrep_t.ap(), in_=wlrep.ap()), prm, 16)
            emit_dma("a", act.dma_start(out=cst_t.ap(), in_=cst.ap()), prm, 16)
            for t in range(TILES):
                # Pin the ach increment order: gate on tail(t-1)'s combined
                # L1 (which requires sq(t-1)); it also covers dv_t since
                # tail(t-1) follows head(t) in the DVE stream. t=0 gates on
                # its own dvb.
                thr = dv_done[t] if t == 0 else s2first_done[t - 1]
                emit_wait("a", act, vch, thr)
                emit("a", act.square(
                    dsa_t[t].ap()[:, HALF:NM], dsa_t[t].ap()[:, 0:HALF]
                ))
                sq_done[t] = emit("a", act.square(
                    dsb_t[t].ap()[:, HALF:NM], dsb_t[t].ap()[:, 0:HALF]
                ))

        @block.sync
        def _(sync):
            def ld(t, h):
                lo, hi = (0, HALF) if h == 0 else (HALF, NM)
                emit_dma(
                    "s",
                    sync.dma_start(
                        out=df_t[t].ap()[:, lo:hi],
                        in_=dense.ap()[128 * t : 128 * (t + 1), lo:hi],
                    ),
                    (lda if h == 0 else ldb)[t], 16,
                )

            # throttle: only one tile's loads queued at a time, so each
            # load's completion semaphore fires as soon as its data lands
            # (a deep queue round-robins packets and delays the first
            # completion to nearly the last)
            ld(0, 0)
            for t in range(1, TILES):
                emit_wait("s", sync, lda[t - 1], 16)
                ld(t, 0)
                ld(t, 1)
            emit_dma(
                "s",
                sync.dma_start(out=out.ap(), in_=o2all.ap()),
                sts, 16,
                wait=(vch, o2_done[3]),
            )
            sync.wait_ge(sts, 16)

    nc.compile()
    return nc


def _get_program():
    if "nc" not in _CACHE:
        _CACHE["nc"] = _build_program()
    return _CACHE["nc"]


def _host_prep(inputs):
    dense = np.ascontiguousarray(
        np.asarray(inputs["dense"], dtype=np.float32).reshape(B, NM)
    )
    v = np.asarray(inputs["v"], dtype=np.float32).reshape(1, NM)
    Wl = np.asarray(inputs["Wl"], dtype=np.float32).reshape(N)
    Wp = np.asarray(inputs["Wp"], dtype=np.float32).reshape(M)
    bl = float(np.asarray(inputs["bl"], dtype=np.float32).reshape(-1)[0])
    bp = float(np.asarray(inputs["bp"], dtype=np.float32).reshape(-1)[0])

    import ml_dtypes

    c = (Wp / (2.0 * P_PAIRS)).astype(np.float32)
    vrep = np.ascontiguousarray(
        np.broadcast_to(v.astype(ml_dtypes.bfloat16), (128, NM))
    )
    crep = np.ascontiguousarray(np.broadcast_to(c[None, :], (128, M)))
    wlrep = np.ascontiguousarray(np.broadcast_to(Wl[None, :], (128, N)))
    cst = np.full((128, 1), bl + bp, dtype=np.float32)

    in_maps = []
    for i in range(NCORES):
        in_maps.append(
            {
                "dense": dense[BS * i : BS * (i + 1)],
                "vrep": vrep,
                "crep": crep,
                "wlrep": wlrep,
                "cst": cst,
            }
        )
    return in_maps


def _gather(res):
    # out[p, t] holds batch row 128*t + p of the core's shard
    outs = []
    for i in range(NCORES):
        arr = np.asarray(res.results[i]["out"], np.float32)  # [128, TILES]
        outs.append(arr.T.reshape(BS))
    return np.concatenate(outs).reshape(B, 1)


def kernel(**inputs) -> np.ndarray:
    from concourse.bass_utils import run_bass_kernel_spmd

    nc = _get_program()
    in_maps = _host_prep(inputs)
    res = run_bass_kernel_spmd(nc, in_maps, core_ids=list(range(NCORES)))
    return _gather(res)
